# revision 24
# baseline (speedup 1.0000x reference)
"""Trainium2 Bass kernel for nn_DAGNessLoss.

Loss = (trace(exp(W0 * W0)) - N)^2 with N = 8192.

trace(exp(W0 ∘ W0)) only touches the diagonal after the elementwise exp,
so the loss reduces exactly to (sum_i exp(W0[i,i]^2) - N)^2.

Sharding (per the row-wise hint): core k owns rows [k*1024, (k+1)*1024);
the only entries of that row-block that contribute to the trace are its
diagonal-block diagonal entries W0[i,i]. Each core receives those 1024
entries (extracted at shard time), computes exp(x^2) on device (DVE
square -> ACT exp), and the 8 per-core result tiles are gathered and
reduced to the final scalar on the host.

Kernel-latency design (cost-model timeline ~4.0us/core; the 4KB
payloads are pure latency, so the kernel is fixed-overhead-bound):
- Input: one HWDGE DMA (SP). Its fixed chain (seq + DGE + ~0.9us
  completion-sem propagation) is the dominant term.
- Output: SWDGE prepare/trigger. The descriptor generation for the
  SBUF->DRAM writeback (kv_writeback with a zero page index == plain
  copy) runs on the Pool Q7 *during* the input DMA; once the exp tile
  lands, a single trigger_dma doorbell fires the pre-built descriptors,
  hiding the ~1.3us DGE setup a plain dma_start would pay serially.
  (Requires codegen_inst_isa_subclasses -- raw Bass skips it and walrus
  then rejects the empty InstISA bytes with "ISA wrong length".)
- An explicit InstLoadActFuncSet(exp_and_others) is ACT's first
  instruction (it has no data operands, so no wait): the ~1.3us exp
  table load runs from t~0 under the input DMA, and walrus does not
  insert a second load before the real Exp (verified in disassembly).
- The Bass-init const-AP memsets, the init/exit all-engine barriers,
  the (unreferenced) per-engine register setup, and all branches are
  stripped from the BIR after tracing (single straight-line stream per
  engine); the bias AP the Exp needs is zeroed by the otherwise-idle
  DVE under a semaphore.
- The completion wait for the triggered writeback is REQUIRED: ending
  the kernel with the transfer still in flight trips
  NRT_EXEC_UNIT_UNRECOVERABLE (measured). It parks on SP, whose
  semaphore-receive path is the cheapest.
- The final partial-sum reduction happens host-side during the unshard.
"""

import numpy as np

import concourse.bass as bass
import concourse.mybir as mybir
from concourse import library_config
from concourse.bass_utils import run_bass_kernel_spmd
from concourse.hw_specs import get_activation_tables
from concourse.library_overlay import lower_extended_insts

N = 8192
N_CORES = 8
BLK = N // N_CORES  # 1024 diagonal entries per core
P = 128  # SBUF partitions
F = BLK // P  # 8 elements per partition

_NC_CACHE = {}


def _build_module(prepared_writeback: bool = True) -> bass.Bass:
    """prepared_writeback=True: output via SWDGE prepare/trigger (fast
    path; needs custom-ISA codegen + the attn ucode library at runtime).
    False: plain HWDGE output DMA on SP — no exotic dependencies, ~1.3us
    slower; used as an automatic fallback if the fast path fails in the
    execution environment."""
    nc = bass.Bass(target_bir_lowering=False)

    d = nc.dram_tensor("d", [P, F], mybir.dt.float32, kind="ExternalInput")
    out = nc.dram_tensor("out", [P, F], mybir.dt.float32, kind="ExternalOutput")

    exp_set_id = list(get_activation_tables("gen3").keys()).index("exp_and_others")

    with (
        nc.Block() as block,
        nc.semaphore("A") as A,  # input DMA completion (16)
        nc.semaphore("C") as C,  # writeback DMA completion (16); SWDGE-owned
        nc.semaphore("B") as B,  # zbias -> 1, ci -> 2, sq -> 3, e -> 4
        nc.semaphore("PR") as PR,  # writeback descriptors committed
        nc.sbuf_tensor("x", [P, F], mybir.dt.float32) as x,
        nc.sbuf_tensor("sq", [P, F], mybir.dt.float32) as sq,
        nc.sbuf_tensor("e", [P, F], mybir.dt.float32) as e,
        nc.sbuf_tensor("zbias", [P, 1], mybir.dt.float32) as zbias,
        nc.sbuf_tensor("ci", [P, 1], mybir.dt.int32) as ci,
    ):

        @block.sync
        def _(sync):
            sync.dma_start(x[:, :], d[:, :]).then_inc(A, 16)
            if prepared_writeback:
                sync.wait_ge(C, 16)  # output landed in DRAM
            else:
                sync.wait_ge(B, 4)  # e written
                sync.dma_start(out[:, :], e[:, :]).then_inc(C, 16)
                sync.wait_ge(C, 16)  # output landed in DRAM

        @block.vector
        def _(vector):
            vector.memset(zbias[:, :], 0.0).then_inc(B, 1)
            vector.memset(ci[:, :], 0).then_inc(B, 1)
            vector.wait_ge(A, 16)
            vector.tensor_mul(sq[:, :], x[:, :], x[:, :]).then_inc(B, 1)

        @block.scalar
        def _(scalar):
            # Explicit exp-table load as ACT's first instruction: no data
            # operands, so it needs no wait and runs under the input DMA.
            scalar.add_instruction(
                mybir.InstLoadActFuncSet(
                    name=nc.get_next_instruction_name(),
                    act_func_set_id=exp_set_id,
                    ins=[],
                    outs=[],
                )
            )
            scalar.wait_ge(B, 3)
            scalar.activation(
                e[:, :],
                sq[:, :],
                mybir.ActivationFunctionType.Exp,
                bias=zbias[:, :],
            ).then_inc(B, 1)

        if prepared_writeback:

            @block.gpsimd
            def _(gpsimd):
                gpsimd.load_library(library_config.attn)
                gpsimd.wait_ge(B, 2)  # ci zeroed
                # View e as [d_head_inner=128, d_head_outer=1, batch=1,
                # ncn=8] and out as [batch=1, dhi=128, dho=1, n_ctx=8];
                # with ctx index 0 this is a plain SBUF->DRAM copy of the
                # [128, 8] tile, but through the prepare/trigger path.
                in_ap = bass.AP(e.tensor if hasattr(e, "tensor") else e, 0,
                                [[F, P], [F, 1], [F, 1], [1, F]])
                out_ap = bass.AP(out, 0, [[P * F, 1], [F, P], [F, 1], [1, F]])
                gpsimd.kv_writeback(
                    out_ap, in_ap, ci[:, :], prepare_only=True, sem=C
                ).then_inc(PR, 1)
                gpsimd.wait_ge(PR, 1)  # descriptors committed to the ring
                gpsimd.wait_ge(B, 4)  # e written
                gpsimd.trigger_dma(1)

    lower_extended_insts(nc)
    return nc


def _strip_overhead(nc: bass.Bass) -> bass.Bass:
    """Collapse the block graph into one straight-line block per engine
    stream, dropping: the Bass-init const-AP memsets, the init/exit
    all-engine drain+barrier chains, the per-engine zero/bounds-check
    register setup, and every branch (each engine starts its stream at
    offset 0 and halts at stream end). Nothing in this kernel depends on
    any of it: no instruction references a register, the only bias AP
    used is zeroed inside the block (under a semaphore), and every
    cross-engine dependency is semaphore-guarded. The final
    wait_ge(C, 16) keeps the output-DMA completion inside the kernel."""
    blocks = list(nc.m.functions[0].blocks)
    merged = []
    for bi, blk in enumerate(blocks):
        for i in blk.instructions:
            if bi == 0 or bi == len(blocks) - 1:
                # entry/exit: keep only the function-entry call marker
                if isinstance(i, mybir.InstCall):
                    merged.append(i)
            elif not isinstance(i, mybir.InstUnconditionalBranch):
                merged.append(i)
    blocks[0].instructions = merged
    for blk in blocks[1:]:
        blk.instructions = []
    return nc


def _get_module(prepared_writeback: bool = True) -> bass.Bass:
    key = prepared_writeback
    if key not in _NC_CACHE:
        _NC_CACHE[key] = _strip_overhead(_build_module(prepared_writeback))
    return _NC_CACHE[key]


_USE_PREPARED = True  # flips to False permanently if the fast path fails


def _run(in_maps):
    global _USE_PREPARED
    if _USE_PREPARED:
        try:
            return run_bass_kernel_spmd(
                _get_module(True), in_maps, core_ids=list(range(N_CORES))
            )
        except Exception:
            # Fast path needs custom-ISA codegen + the attn ucode library;
            # fall back to the dependency-free HWDGE output permanently.
            _USE_PREPARED = False
    return run_bass_kernel_spmd(
        _get_module(False), in_maps, core_ids=list(range(N_CORES))
    )


def kernel(W0: np.ndarray) -> np.ndarray:
    W0 = np.asarray(W0)
    if W0.ndim == 3 and W0.shape[2] == 1:
        W0 = W0[:, :, 0]
    assert W0.shape == (N, N), W0.shape

    # Shard: core k gets the diagonal entries of its row-block.
    diag = np.ascontiguousarray(np.diagonal(W0)).astype(np.float32, copy=False)
    in_maps = [
        {"d": np.ascontiguousarray(diag[k * BLK : (k + 1) * BLK].reshape(P, F))}
        for k in range(N_CORES)
    ]

    res = _run(in_maps)

    # Gather/unshard: reduce the 8 per-core exp tiles.
    tr = 0.0
    for r in res.results:
        tr += float(r["out"].astype(np.float64).sum())
    loss = (tr - float(N)) ** 2.0
    return np.array(loss, dtype=np.float32)
